# revision 4
# baseline (speedup 1.0000x reference)
"""Trainium2 Bass kernel for a 2-layer GCN (PyG GCNConv semantics).

Strategy (8 NeuronCores, SPMD, full I/O):
  - Host folds symmetric deg^-1/2 normalization + edge weight into one
    per-edge scalar w~ = dinv[src]*w*dinv[dst]; self-loops become one
    extra "tile" per dst block (sequential z load, w~ = dinv^2).
  - Destinations partitioned contiguously: 8 cores x 98 blocks x 128.
    Sources split into 4 groups so int16 indices work with dma_gather;
    the group windows are 32768 rows (int16 range) vs 25088-row spacing,
    so groups 1-3 can shed overflow edges to the previous group. Static
    per-block tile caps (6,4,4,4) then hold w.h.p., cutting gather
    padding vs. a uniform cap. Cells are padded with dummy index-0 rows
    (weight 0) so gathers have NO -1 skips and NO runtime counts, which
    lets 7 blocks share one dma_gather call per group (amortizes the
    ~1us SWDGE descriptor-generation cost on the Pool engine that
    dominated the previous version).
  - The one-hot scatter matrices are built ON DEVICE from a compact
    (slot, w) stream (4 bytes/edge-slot instead of 256): per block, two
    wide DVE ops over [128, 19*128] fp16:
        S = (iota == slot_bcast) * w_bcast
    using step-0 broadcast access patterns; this removes the 67MB/layer
    one-hot stream from DRAM that previously half-filled the DMA bus.
  - Per dst block: PSUM aggT[f, n] += G_t.T @ S_t over 19 tiles
    (TensorE fp16, fp32 accum), then out = relu(aggT.T @ W + b).
  - Two launches (one per GCN layer) of the same compiled program.
"""

import os
from contextlib import ExitStack

import numpy as np

import concourse.bacc as bacc
import concourse.bass as bass
import concourse.mybir as mybir
import concourse.tile as tile
from concourse import bass_utils

P = 128
D = 128
NCORES = 8
NGROUP = 4
N_NODES = 100000
NB_PER_CORE = 98
SHARD = NB_PER_CORE * P          # 12544
N_PAD = SHARD * NCORES           # 100352
GROWS = N_PAD // NGROUP          # 25088 group spacing
WINDOW = 1 << 15                 # 32768 int16-addressable rows per group
ZROWS = (NGROUP - 1) * GROWS + WINDOW   # padded z rows: 108032
KBLK = 7                         # dst blocks per dma_gather call
NCALL = NB_PER_CORE // KBLK      # 14
CAPS = (6, 4, 4, 4)              # gather tiles per (block, group)
TB = sum(CAPS)                   # 18 gathered tiles per block
TB1 = TB + 1                     # + self tile
TBASE = (0, CAPS[0], CAPS[0] + CAPS[1], CAPS[0] + CAPS[1] + CAPS[2])

_nc_cache = {}


def build_nc():
    dt = mybir.dt
    nblk = NB_PER_CORE
    ixcols = KBLK * TB * 8                 # idx cols per call (16-wrapped)
    swcols = KBLK * 2 * TB1                # slot+w cols per call
    nc = bacc.Bacc(
        "TRN2",
        target_bir_lowering=False,
        debug=False,
        enable_asserts=False,
        num_devices=1,
        num_swdge_queues=4,
    )
    zt = nc.dram_tensor("zt", [ZROWS, D], dt.float16, kind="ExternalInput")
    ixd = nc.dram_tensor("ixd", [NCALL, P, ixcols], dt.int16,
                         kind="ExternalInput")
    swd = nc.dram_tensor("swd", [NCALL, P, swcols], dt.float16,
                         kind="ExternalInput")
    zself = nc.dram_tensor("zself", [nblk * P, D], dt.float16,
                           kind="ExternalInput")
    iot = nc.dram_tensor("iot", [P, TB1 * P], dt.float16, kind="ExternalInput")
    wt = nc.dram_tensor("wt", [D, D], dt.float16, kind="ExternalInput")
    brow = nc.dram_tensor("brow", [1, D], dt.float16, kind="ExternalInput")
    out = nc.dram_tensor("out", [nblk * P, D], dt.float32,
                         kind="ExternalOutput")

    with tile.TileContext(nc) as tc, ExitStack() as ctx:
        const = ctx.enter_context(tc.tile_pool(name="const", bufs=1))
        meta = ctx.enter_context(tc.tile_pool(name="meta", bufs=2))
        swp = ctx.enter_context(tc.tile_pool(name="swp", bufs=2))
        zsp = ctx.enter_context(tc.tile_pool(name="zsp", bufs=2))
        gpools = [
            ctx.enter_context(tc.tile_pool(name=f"g{g}", bufs=2))
            for g in range(NGROUP)
        ]
        spool = ctx.enter_context(tc.tile_pool(name="s", bufs=4))
        apool = ctx.enter_context(tc.tile_pool(name="agg", bufs=3))
        opool = ctx.enter_context(tc.tile_pool(name="o", bufs=2))
        ppool = ctx.enter_context(tc.tile_pool(name="ps", bufs=2, space="PSUM"))
        p2pool = ctx.enter_context(tc.tile_pool(name="ps2", bufs=2,
                                                space="PSUM"))

        w_t = const.tile([D, D], dt.float16)
        nc.sync.dma_start(out=w_t[:], in_=wt[:])
        b_t = const.tile([1, D], dt.float16)
        nc.sync.dma_start(out=b_t[:], in_=brow[:])
        ones_t = const.tile([1, P], dt.float16)
        nc.vector.memset(ones_t[:], 1.0)
        iota_t = const.tile([P, TB1 * P], dt.float16)
        nc.sync.dma_start(out=iota_t[:], in_=iot[:])
        iota3 = iota_t[:].rearrange("p (t n) -> p t n", n=P)

        for c in range(NCALL):
            ix = meta.tile([P, ixcols], dt.int16, tag="ix")
            nc.sync.dma_start(out=ix[:], in_=ixd[c])
            sw_t = swp.tile([P, swcols], dt.float16, tag="sw")
            nc.scalar.dma_start(out=sw_t[:], in_=swd[c])
            zs = zsp.tile([P, KBLK * D], dt.float16, tag="zs")
            nc.sync.dma_start(
                out=zs[:].rearrange("p (k d) -> p k d", d=D),
                in_=zself[c * KBLK * P:(c + 1) * KBLK * P, :].rearrange(
                    "(k p) d -> p k d", p=P),
            )

            g_ws = []
            col0 = 0
            for g in range(NGROUP):
                gcols = KBLK * CAPS[g] * 8
                nidx = KBLK * CAPS[g] * P
                g_w = gpools[g].tile([P, KBLK * CAPS[g] * P], dt.float16,
                                     tag=f"G{g}")
                nc.gpsimd.dma_gather(
                    out_ap=g_w[:].rearrange("p (j n) -> p j n", n=P),
                    in_ap=zt[g * GROWS:g * GROWS + WINDOW, :],
                    idxs_ap=ix[:, col0:col0 + gcols],
                    num_idxs=nidx,
                    num_idxs_reg=nidx,
                    elem_size=P,
                    queue_num=g,
                    single_packet=False,
                )
                g_ws.append(g_w)
                col0 += gcols

            ost = opool.tile([P, KBLK * D], dt.float32, tag="o")
            for k in range(KBLK):
                s_t = spool.tile([P, TB1 * P], dt.float16, tag="S")
                s3 = s_t[:].rearrange("p (t n) -> p t n", n=P)
                slot_b = sw_t[:, k * 2 * TB1:k * 2 * TB1 + TB1, None] \
                    .broadcast_to((P, TB1, P))
                w_b = sw_t[:, k * 2 * TB1 + TB1:(k + 1) * 2 * TB1, None] \
                    .broadcast_to((P, TB1, P))
                nc.vector.tensor_tensor(
                    out=s3, in0=iota3, in1=slot_b,
                    op=mybir.AluOpType.is_equal)
                nc.vector.tensor_tensor(
                    out=s3, in0=s3, in1=w_b, op=mybir.AluOpType.mult)

                psum = ppool.tile([P, P], dt.float32, tag="psA")
                t = 0
                for g in range(NGROUP):
                    for j in range(CAPS[g]):
                        nc.tensor.matmul(
                            out=psum[:],
                            lhsT=g_ws[g][:, (k * CAPS[g] + j) * P:
                                         (k * CAPS[g] + j + 1) * P],
                            rhs=s_t[:, t * P:(t + 1) * P],
                            start=(t == 0),
                            stop=False,
                        )
                        t += 1
                nc.tensor.matmul(
                    out=psum[:],
                    lhsT=zs[:, k * D:(k + 1) * D],
                    rhs=s_t[:, TB * P:TB1 * P],
                    start=False, stop=True,
                )

                agg_t = apool.tile([P, P], dt.float16, tag="aggT")
                nc.scalar.activation(out=agg_t[:], in_=psum[:],
                                     func=mybir.ActivationFunctionType.Copy)

                psum2 = p2pool.tile([P, D], dt.float32, tag="psB")
                nc.tensor.matmul(out=psum2[:], lhsT=agg_t[:], rhs=w_t[:],
                                 start=True, stop=False)
                nc.tensor.matmul(out=psum2[:], lhsT=ones_t[:], rhs=b_t[:],
                                 start=False, stop=True)
                nc.scalar.activation(out=ost[:, k * D:(k + 1) * D],
                                     in_=psum2[:],
                                     func=mybir.ActivationFunctionType.Relu)

            nc.sync.dma_start(
                out=out[c * KBLK * P:(c + 1) * KBLK * P, :].rearrange(
                    "(k p) d -> p k d", p=P),
                in_=ost[:].rearrange("p (k d) -> p k d", d=D),
            )

    nc.compile()
    return nc


def preprocess(src, dst, ew):
    """Host-side: edge->group assignment with overflow cascade, padded
    per-cell gather index streams, and compact (slot, w) scatter metadata.

    Returns (ixd, swd, zself_w) with
      ixd: [NCORES, NCALL, P, KBLK*TB*8] int16 wrapped gather indices
      swd: [NCORES, NCALL, P, KBLK*2*TB1] fp16 slot/weight columns
    """
    nblocks = NCORES * NB_PER_CORE
    deg = np.bincount(dst, weights=ew.astype(np.float64),
                      minlength=N_NODES) + 1.0
    dinv = (1.0 / np.sqrt(deg)).astype(np.float32)
    wtil = (dinv[src] * ew.astype(np.float32) * dinv[dst]).astype(np.float32)
    wself = np.zeros(N_PAD, np.float32)
    wself[:N_NODES] = dinv * dinv

    blk = (dst // P).astype(np.int64)
    ng = (src // GROWS).astype(np.int64)
    zone = (src % GROWS) < (WINDOW - GROWS)
    elig = zone & (ng >= 1)

    cellng = blk * NGROUP + ng
    n = np.bincount(cellng, minlength=nblocks * NGROUP) \
        .reshape(nblocks, NGROUP)
    e = np.bincount(cellng[elig], minlength=nblocks * NGROUP) \
        .reshape(nblocks, NGROUP)

    caps = np.array(CAPS) * P
    shed = np.zeros((nblocks, NGROUP), np.int64)
    load = n[:, 3]
    for g in (3, 2, 1):
        shed[:, g] = np.maximum(load - caps[g], 0)
        if not (shed[:, g] <= e[:, g]).all():
            raise RuntimeError("group shed infeasible; raise CAPS")
        load = n[:, g - 1] + shed[:, g]
    if not (load <= caps[0]).all():
        raise RuntimeError("group 0 overflow; raise CAPS")

    # shed the smallest-src eligible edges of each (block, group) cell
    order = np.lexsort((src, ~elig, cellng))
    starts = np.zeros(nblocks * NGROUP, np.int64)
    counts = n.reshape(-1)
    np.cumsum(counts[:-1], out=starts[1:])
    rank = np.arange(len(src)) - starts[cellng[order]]
    shed_sorted = rank < shed.reshape(-1)[cellng[order]]
    ag = ng.copy()
    ag[order[shed_sorted]] -= 1

    # final cells, sorted by (cell, src); position within cell
    cell = blk * NGROUP + ag
    order2 = np.lexsort((src, cell))
    cell_s = cell[order2]
    src_s = src[order2]
    dst_s = dst[order2]
    w_s = wtil[order2]
    counts2 = np.bincount(cell_s, minlength=nblocks * NGROUP)
    starts2 = np.zeros(nblocks * NGROUP, np.int64)
    np.cumsum(counts2[:-1], out=starts2[1:])
    pos = np.arange(len(src_s)) - starts2[cell_s]

    ag_s = cell_s % NGROUP
    blk_s = cell_s // NGROUP
    i16 = (src_s - ag_s * GROWS).astype(np.int16)

    # padded per-group index arrays [nblocks, cap_rows]
    idxpads = []
    for g in range(NGROUP):
        m = ag_s == g
        a = np.zeros((nblocks, caps[g]), np.int16)
        a[blk_s[m], pos[m]] = i16[m]
        idxpads.append(a)

    # slot/w columns [nblocks, P, TB1]
    slot = np.zeros((nblocks, P, TB1), np.float16)
    warr = np.zeros((nblocks, P, TB1), np.float16)
    tcol = np.take(TBASE, ag_s) + pos // P
    prow = pos % P
    slot[blk_s, prow, tcol] = (dst_s % P).astype(np.float16)
    warr[blk_s, prow, tcol] = w_s.astype(np.float16)
    slot[:, :, TB] = np.arange(P, dtype=np.float16)[None, :]
    warr[:, :, TB] = wself.astype(np.float16).reshape(nblocks, P)

    # wrap indices: per (core, call, group): [KBLK*cap] -> [128, KBLK*cap/16]
    ixparts = []
    for g in range(NGROUP):
        a = idxpads[g].reshape(NCORES, NCALL, KBLK * caps[g])
        a = a.reshape(NCORES, NCALL, KBLK * caps[g] // 16, 16)
        a = a.transpose(0, 1, 3, 2)                   # [8, 14, 16, cols]
        a = np.tile(a, (1, 1, 8, 1))                  # [8, 14, 128, cols]
        ixparts.append(a)
    ixd = np.ascontiguousarray(np.concatenate(ixparts, axis=3))

    sw = np.concatenate([slot, warr], axis=2)         # [nblocks, P, 2*TB1]
    sw = sw.reshape(NCORES, NCALL, KBLK, P, 2 * TB1)
    swd = np.ascontiguousarray(sw.transpose(0, 1, 3, 2, 4).reshape(
        NCORES, NCALL, P, KBLK * 2 * TB1))
    return ixd, swd


def run_layer(nc, z_f16, ixd, swd, W, b, *, trace=False, tmpdir=None):
    iot = np.tile(np.arange(P, dtype=np.float16), (P, TB1)) \
        .reshape(P, TB1 * P)
    in_maps = []
    for c in range(NCORES):
        in_maps.append({
            "zt": z_f16,
            "zself": z_f16[c * SHARD:(c + 1) * SHARD],
            "ixd": ixd[c],
            "swd": swd[c],
            "iot": iot,
            "wt": np.ascontiguousarray(W.astype(np.float16)),
            "brow": np.ascontiguousarray(
                b.astype(np.float16).reshape(1, D)),
        })
    res = bass_utils.run_bass_kernel_spmd(
        nc, in_maps, core_ids=list(range(NCORES)), trace=trace, tmpdir=tmpdir,
    )
    out = np.concatenate([res.results[c]["out"] for c in range(NCORES)],
                         axis=0)
    return out, res


def _enable_tracing():
    """Install the NTFF profile hook that this image's antenv lacks, and
    neuter the artifact upload (no bucket access here)."""
    import sys
    import types
    try:
        import antenv.axon_hooks  # noqa: F401
        have = True
    except ImportError:
        have = False
    if not have:
        mod = types.ModuleType("antenv.axon_hooks")
        mod._hook = None

        def set_axon_ntff_profile_hook(h):
            mod._hook = h

        def get_axon_ntff_profile_hook():
            return mod._hook

        mod.set_axon_ntff_profile_hook = set_axon_ntff_profile_hook
        mod.get_axon_ntff_profile_hook = get_axon_ntff_profile_hook
        sys.modules["antenv.axon_hooks"] = mod
        from trn_agent_boot.trn_boot import _ntff_profile_via_ctypes
        hook = _ntff_profile_via_ctypes("/opt/axon/libaxon_pjrt.so")
        mod.set_axon_ntff_profile_hook(hook)
    bass_utils.upload_artifacts = lambda tmpdir: f"local:{tmpdir}"


def kernel(x, edge_index, edge_weight, W1, b1, W2, b2):
    x = np.asarray(x, dtype=np.float32)
    edge_index = np.asarray(edge_index)
    edge_weight = np.asarray(edge_weight, dtype=np.float32)
    src = edge_index[0].astype(np.int64)
    dst = edge_index[1].astype(np.int64)

    ixd, swd = preprocess(src, dst, edge_weight)

    if "nc" not in _nc_cache:
        _nc_cache["nc"] = build_nc()
    nc = _nc_cache["nc"]

    trace = bool(int(os.environ.get("GCN_TRACE", "0")))
    if trace:
        _enable_tracing()

    z1 = np.zeros((ZROWS, D), np.float16)
    z1[:N_NODES] = x.astype(np.float16)
    h1, res1 = run_layer(nc, z1, ixd, swd, W1, b1, trace=trace)

    z2 = np.zeros((ZROWS, D), np.float16)
    z2[:N_PAD] = h1.astype(np.float16)
    h2, res2 = run_layer(nc, z2, ixd, swd, W2, b2, trace=trace)

    if trace:
        t1 = res1.exec_time_ns or 0
        t2 = res2.exec_time_ns or 0
        print(f"[kernel] layer1 exec: {t1} ns, layer2 exec: {t2} ns, "
              f"total: {t1 + t2} ns")
        kernel.last_exec_ns = t1 + t2
        kernel.last_results = (res1, res2)

    return h2[:N_NODES].astype(np.float32)


# revision 11
# speedup vs baseline: 1.0811x; 1.0811x over previous
"""Trainium2 Bass kernel for a 2-layer GCN (PyG GCNConv semantics).

Strategy (8 NeuronCores, SPMD, full I/O):
  - Host folds symmetric deg^-1/2 normalization + edge weight into one
    per-edge scalar w~ = dinv[src]*w*dinv[dst]; self-loops become one
    extra "tile" per dst block (sequential z load, w~ = dinv^2).
  - Destinations partitioned contiguously: 8 cores x 98 blocks x 128.
    Sources split into 4 groups so int16 indices work with dma_gather;
    the group windows are 32768 rows (int16 range) vs 25088-row spacing,
    so groups 1-3 can shed overflow edges to the previous group. Static
    per-block tile caps (6,4,4,4) then hold w.h.p., cutting gather
    padding vs. a uniform cap. Cells are padded with dummy index-0 rows
    (weight 0) so gathers have NO -1 skips and NO runtime counts, which
    lets 7 blocks share one dma_gather call per group (amortizes the
    ~1us SWDGE descriptor-generation cost on the Pool engine that
    dominated the previous version).
  - The one-hot scatter matrices are built ON DEVICE from a compact
    (slot, w) stream (4 bytes/edge-slot instead of 256): per block, two
    wide DVE ops over [128, 19*128] fp16:
        S = (iota == slot_bcast) * w_bcast
    using step-0 broadcast access patterns; this removes the 67MB/layer
    one-hot stream from DRAM that previously half-filled the DMA bus.
  - Per dst block: PSUM aggT[f, n] += G_t.T @ S_t over 19 tiles
    (TensorE fp16, fp32 accum), then out = relu(aggT.T @ W + b).
  - Two launches (one per GCN layer) of the same compiled program.
"""

import os
from contextlib import ExitStack

import numpy as np

import concourse.bacc as bacc
import concourse.bass as bass
import concourse.mybir as mybir
import concourse.tile as tile
from concourse import bass_utils

P = 128
D = 128
NCORES = 8
NGROUP = 4
N_NODES = 100000
NB_PER_CORE = 98
SHARD = NB_PER_CORE * P          # 12544
N_PAD = SHARD * NCORES           # 100352
GROWS = N_PAD // NGROUP          # 25088 group spacing
WINDOW = 1 << 15                 # 32768 int16-addressable rows per group
ZROWS = (NGROUP - 1) * GROWS + WINDOW   # padded z rows: 108032
KBLK = 7                         # dst blocks per dma_gather call
NCALL = NB_PER_CORE // KBLK      # 14
CAPS = (5, 4, 4, 4)              # gather tiles per (block, group)
CAPS_FALLBACK = (6, 4, 4, 4)
PREP = bool(int(os.environ.get("GCN_PREP", "1")))

_nc_cache = {}


def build_nc(caps, prep):
    dt = mybir.dt
    TB = sum(caps)
    TB1 = TB + 1
    nblk = NB_PER_CORE
    ixcols = KBLK * TB * 8                 # idx cols per call (16-wrapped)
    swcols = KBLK * 2 * TB1                # slot+w cols per call
    nc = bacc.Bacc(
        "TRN2",
        target_bir_lowering=False,
        debug=False,
        enable_asserts=False,
        num_devices=1,
        num_swdge_queues=4,
    )
    zt = nc.dram_tensor("zt", [ZROWS, D], dt.float16, kind="ExternalInput")
    ixd = nc.dram_tensor("ixd", [NCALL, P, ixcols], dt.int16,
                         kind="ExternalInput")
    swd = nc.dram_tensor("swd", [NCALL, P, swcols], dt.float16,
                         kind="ExternalInput")
    zself = nc.dram_tensor("zself", [nblk * P, D], dt.float16,
                           kind="ExternalInput")
    iot = nc.dram_tensor("iot", [P, TB1 * P], dt.float16, kind="ExternalInput")
    wt = nc.dram_tensor("wt", [D, D], dt.float16, kind="ExternalInput")
    brow = nc.dram_tensor("brow", [1, D], dt.float16, kind="ExternalInput")
    out = nc.dram_tensor("out", [nblk * P, D], dt.float32,
                         kind="ExternalOutput")

    with tile.TileContext(nc) as tc, ExitStack() as ctx:
        const = ctx.enter_context(tc.tile_pool(name="const", bufs=1))
        meta = ctx.enter_context(tc.tile_pool(name="meta", bufs=3))
        swp = ctx.enter_context(tc.tile_pool(name="swp", bufs=3))
        zsp = ctx.enter_context(tc.tile_pool(name="zsp", bufs=3))
        gpools = [
            ctx.enter_context(tc.tile_pool(name=f"g{g}", bufs=4))
            for g in range(NGROUP)
        ]
        spool = ctx.enter_context(tc.tile_pool(name="s", bufs=4))
        apool = ctx.enter_context(tc.tile_pool(name="agg", bufs=3))
        opool = ctx.enter_context(tc.tile_pool(name="o", bufs=2))
        ppool = ctx.enter_context(tc.tile_pool(name="ps", bufs=3, space="PSUM"))
        p2pool = ctx.enter_context(tc.tile_pool(name="ps2", bufs=2,
                                                space="PSUM"))

        w_t = const.tile([D, D], dt.float16)
        nc.sync.dma_start(out=w_t[:], in_=wt[:])
        b_t = const.tile([1, D], dt.float16)
        nc.sync.dma_start(out=b_t[:], in_=brow[:])
        ones_t = const.tile([1, P], dt.float16)
        nc.vector.memset(ones_t[:], 1.0)
        iota_t = const.tile([P, TB1 * P], dt.float16)
        nc.sync.dma_start(out=iota_t[:], in_=iot[:])
        iota3 = iota_t[:].rearrange("p (t n) -> p t n", n=P)

        for c in range(NCALL):
            ix = meta.tile([P, ixcols], dt.int16, tag="ix")
            nc.sync.dma_start(out=ix[:], in_=ixd[c])
            sw_t = swp.tile([P, swcols], dt.float16, tag="sw")
            nc.scalar.dma_start(out=sw_t[:], in_=swd[c])
            zs = zsp.tile([P, KBLK * D], dt.float16, tag="zs")
            nc.sync.dma_start(
                out=zs[:].rearrange("p (k d) -> p k d", d=D),
                in_=zself[c * KBLK * P:(c + 1) * KBLK * P, :].rearrange(
                    "(k p) d -> p k d", p=P),
            )

            g_ws = []
            col0 = 0
            for g in range(NGROUP):
                gcols = KBLK * caps[g] * 8
                nidx = KBLK * caps[g] * P
                g_w = gpools[g].tile([P, KBLK * caps[g] * P], dt.float16,
                                     tag=f"G{g}")
                kw = (dict(prepare_only=True,
                           sem=nc.alloc_semaphore(f"gq{c}_{g}"))
                      if prep else {})
                nc.gpsimd.dma_gather(
                    out_ap=g_w[:].rearrange("p (j n) -> p j n", n=P),
                    in_ap=zt[g * GROWS:g * GROWS + WINDOW, :],
                    idxs_ap=ix[:, col0:col0 + gcols],
                    num_idxs=nidx,
                    num_idxs_reg=nidx,
                    elem_size=P,
                    queue_num=g,
                    single_packet=False,
                    **kw,
                )
                g_ws.append(g_w)
                col0 += gcols
            if prep:
                for g in range(NGROUP):
                    nc.gpsimd.trigger_dma(count=None, queue_num=g)

            ost = opool.tile([P, KBLK * D], dt.float32, tag="o")
            for k in range(KBLK):
                s_t = spool.tile([P, TB1 * P], dt.float16, tag="S")
                s3 = s_t[:].rearrange("p (t n) -> p t n", n=P)
                slot_b = sw_t[:, k * 2 * TB1:k * 2 * TB1 + TB1, None] \
                    .broadcast_to((P, TB1, P))
                w_b = sw_t[:, k * 2 * TB1 + TB1:(k + 1) * 2 * TB1, None] \
                    .broadcast_to((P, TB1, P))
                nc.vector.tensor_tensor(
                    out=s3, in0=iota3, in1=slot_b,
                    op=mybir.AluOpType.is_equal)
                nc.vector.tensor_tensor(
                    out=s3, in0=s3, in1=w_b, op=mybir.AluOpType.mult)

                psum = ppool.tile([P, P], dt.float32, tag="psA")
                t = 0
                for g in range(NGROUP):
                    for j in range(caps[g]):
                        nc.tensor.matmul(
                            out=psum[:],
                            lhsT=g_ws[g][:, (k * caps[g] + j) * P:
                                         (k * caps[g] + j + 1) * P],
                            rhs=s_t[:, t * P:(t + 1) * P],
                            start=(t == 0),
                            stop=False,
                        )
                        t += 1
                nc.tensor.matmul(
                    out=psum[:],
                    lhsT=zs[:, k * D:(k + 1) * D],
                    rhs=s_t[:, TB * P:TB1 * P],
                    start=False, stop=True,
                )

                agg_t = apool.tile([P, P], dt.float16, tag="aggT")
                nc.scalar.activation(out=agg_t[:], in_=psum[:],
                                     func=mybir.ActivationFunctionType.Copy)

                psum2 = p2pool.tile([P, D], dt.float32, tag="psB")
                nc.tensor.matmul(out=psum2[:], lhsT=agg_t[:], rhs=w_t[:],
                                 start=True, stop=False)
                nc.tensor.matmul(out=psum2[:], lhsT=ones_t[:], rhs=b_t[:],
                                 start=False, stop=True)
                nc.scalar.activation(out=ost[:, k * D:(k + 1) * D],
                                     in_=psum2[:],
                                     func=mybir.ActivationFunctionType.Relu)

            nc.sync.dma_start(
                out=out[c * KBLK * P:(c + 1) * KBLK * P, :].rearrange(
                    "(k p) d -> p k d", p=P),
                in_=ost[:].rearrange("p (k d) -> p k d", d=D),
            )

    nc.compile()
    return nc


def preprocess(src, dst, ew, capst):
    """Host-side: edge->group assignment with overflow cascade, padded
    per-cell gather index streams, and compact (slot, w) scatter metadata.

    Returns (ixd, swd) with
      ixd: [NCORES, NCALL, P, KBLK*TB*8] int16 wrapped gather indices
      swd: [NCORES, NCALL, P, KBLK*2*TB1] fp16 slot/weight columns
    """
    TB = sum(capst)
    TB1 = TB + 1
    TBASE = (0, capst[0], capst[0] + capst[1], capst[0] + capst[1] + capst[2])
    nblocks = NCORES * NB_PER_CORE
    deg = np.bincount(dst, weights=ew.astype(np.float64),
                      minlength=N_NODES) + 1.0
    dinv = (1.0 / np.sqrt(deg)).astype(np.float32)
    wtil = (dinv[src] * ew.astype(np.float32) * dinv[dst]).astype(np.float32)
    wself = np.zeros(N_PAD, np.float32)
    wself[:N_NODES] = dinv * dinv

    blk = (dst // P).astype(np.int64)
    ng = (src // GROWS).astype(np.int64)
    zone = (src % GROWS) < (WINDOW - GROWS)
    elig = zone & (ng >= 1)

    cellng = blk * NGROUP + ng
    n = np.bincount(cellng, minlength=nblocks * NGROUP) \
        .reshape(nblocks, NGROUP)
    e = np.bincount(cellng[elig], minlength=nblocks * NGROUP) \
        .reshape(nblocks, NGROUP)

    caps = np.array(capst) * P
    shed = np.zeros((nblocks, NGROUP), np.int64)
    load = n[:, 3]
    for g in (3, 2, 1):
        shed[:, g] = np.maximum(load - caps[g], 0)
        if not (shed[:, g] <= e[:, g]).all():
            raise RuntimeError("group shed infeasible; raise CAPS")
        load = n[:, g - 1] + shed[:, g]
    if not (load <= caps[0]).all():
        raise RuntimeError("group 0 overflow; raise CAPS")

    # shed the smallest-src eligible edges of each (block, group) cell
    order = np.lexsort((src, ~elig, cellng))
    starts = np.zeros(nblocks * NGROUP, np.int64)
    counts = n.reshape(-1)
    np.cumsum(counts[:-1], out=starts[1:])
    rank = np.arange(len(src)) - starts[cellng[order]]
    shed_sorted = rank < shed.reshape(-1)[cellng[order]]
    ag = ng.copy()
    ag[order[shed_sorted]] -= 1

    # final cells, sorted by (cell, src); position within cell
    cell = blk * NGROUP + ag
    order2 = np.lexsort((src, cell))
    cell_s = cell[order2]
    src_s = src[order2]
    dst_s = dst[order2]
    w_s = wtil[order2]
    counts2 = np.bincount(cell_s, minlength=nblocks * NGROUP)
    starts2 = np.zeros(nblocks * NGROUP, np.int64)
    np.cumsum(counts2[:-1], out=starts2[1:])
    pos = np.arange(len(src_s)) - starts2[cell_s]

    ag_s = cell_s % NGROUP
    blk_s = cell_s // NGROUP
    i16 = (src_s - ag_s * GROWS).astype(np.int16)

    # padded per-group index arrays [nblocks, cap_rows]
    idxpads = []
    for g in range(NGROUP):
        m = ag_s == g
        a = np.zeros((nblocks, caps[g]), np.int16)
        a[blk_s[m], pos[m]] = i16[m]
        idxpads.append(a)

    # slot/w columns [nblocks, P, TB1]
    slot = np.zeros((nblocks, P, TB1), np.float16)
    warr = np.zeros((nblocks, P, TB1), np.float16)
    tcol = np.take(TBASE, ag_s) + pos // P
    prow = pos % P
    slot[blk_s, prow, tcol] = (dst_s % P).astype(np.float16)
    warr[blk_s, prow, tcol] = w_s.astype(np.float16)
    slot[:, :, TB] = np.arange(P, dtype=np.float16)[None, :]
    warr[:, :, TB] = wself.astype(np.float16).reshape(nblocks, P)

    # wrap indices: per (core, call, group): [KBLK*cap] -> [128, KBLK*cap/16]
    ixparts = []
    for g in range(NGROUP):
        a = idxpads[g].reshape(NCORES, NCALL, KBLK * caps[g])
        a = a.reshape(NCORES, NCALL, KBLK * caps[g] // 16, 16)
        a = a.transpose(0, 1, 3, 2)                   # [8, 14, 16, cols]
        a = np.tile(a, (1, 1, 8, 1))                  # [8, 14, 128, cols]
        ixparts.append(a)
    ixd = np.ascontiguousarray(np.concatenate(ixparts, axis=3))

    sw = np.concatenate([slot, warr], axis=2)         # [nblocks, P, 2*TB1]
    sw = sw.reshape(NCORES, NCALL, KBLK, P, 2 * TB1)
    swd = np.ascontiguousarray(sw.transpose(0, 1, 3, 2, 4).reshape(
        NCORES, NCALL, P, KBLK * 2 * TB1))
    return ixd, swd


def run_layer(nc, z_f16, ixd, swd, W, b, tb1, *, trace=False, tmpdir=None):
    iot = np.tile(np.arange(P, dtype=np.float16), (P, tb1)) \
        .reshape(P, tb1 * P)
    in_maps = []
    for c in range(NCORES):
        in_maps.append({
            "zt": z_f16,
            "zself": z_f16[c * SHARD:(c + 1) * SHARD],
            "ixd": ixd[c],
            "swd": swd[c],
            "iot": iot,
            "wt": np.ascontiguousarray(W.astype(np.float16)),
            "brow": np.ascontiguousarray(
                b.astype(np.float16).reshape(1, D)),
        })
    res = bass_utils.run_bass_kernel_spmd(
        nc, in_maps, core_ids=list(range(NCORES)), trace=trace, tmpdir=tmpdir,
    )
    out = np.concatenate([res.results[c]["out"] for c in range(NCORES)],
                         axis=0)
    return out, res


def _enable_tracing():
    """Install the NTFF profile hook that this image's antenv lacks, and
    neuter the artifact upload (no bucket access here)."""
    import sys
    import types
    try:
        import antenv.axon_hooks  # noqa: F401
        have = True
    except ImportError:
        have = False
    if not have:
        mod = types.ModuleType("antenv.axon_hooks")
        mod._hook = None

        def set_axon_ntff_profile_hook(h):
            mod._hook = h

        def get_axon_ntff_profile_hook():
            return mod._hook

        mod.set_axon_ntff_profile_hook = set_axon_ntff_profile_hook
        mod.get_axon_ntff_profile_hook = get_axon_ntff_profile_hook
        sys.modules["antenv.axon_hooks"] = mod
        from trn_agent_boot.trn_boot import _ntff_profile_via_ctypes
        hook = _ntff_profile_via_ctypes("/opt/axon/libaxon_pjrt.so")
        mod.set_axon_ntff_profile_hook(hook)
    bass_utils.upload_artifacts = lambda tmpdir: f"local:{tmpdir}"


def kernel(x, edge_index, edge_weight, W1, b1, W2, b2):
    x = np.asarray(x, dtype=np.float32)
    edge_index = np.asarray(edge_index)
    edge_weight = np.asarray(edge_weight, dtype=np.float32)
    src = edge_index[0].astype(np.int64)
    dst = edge_index[1].astype(np.int64)

    try:
        capst = CAPS
        ixd, swd = preprocess(src, dst, edge_weight, capst)
    except RuntimeError:
        capst = CAPS_FALLBACK
        ixd, swd = preprocess(src, dst, edge_weight, capst)
    tb1 = sum(capst) + 1

    key = (capst, PREP)
    if key not in _nc_cache:
        _nc_cache[key] = build_nc(capst, PREP)
    nc = _nc_cache[key]

    trace = bool(int(os.environ.get("GCN_TRACE", "0")))
    if trace:
        _enable_tracing()

    z1 = np.zeros((ZROWS, D), np.float16)
    z1[:N_NODES] = x.astype(np.float16)
    h1, res1 = run_layer(nc, z1, ixd, swd, W1, b1, tb1, trace=trace)

    z2 = np.zeros((ZROWS, D), np.float16)
    z2[:N_PAD] = h1.astype(np.float16)
    h2, res2 = run_layer(nc, z2, ixd, swd, W2, b2, tb1, trace=trace)

    if trace:
        t1 = res1.exec_time_ns or 0
        t2 = res2.exec_time_ns or 0
        print(f"[kernel] layer1 exec: {t1} ns, layer2 exec: {t2} ns, "
              f"total: {t1 + t2} ns")
        kernel.last_exec_ns = t1 + t2
        kernel.last_results = (res1, res2)

    return h2[:N_NODES].astype(np.float32)


# revision 13
# speedup vs baseline: 1.0894x; 1.0077x over previous
"""Trainium2 Bass kernel for a 2-layer GCN (PyG GCNConv semantics).

Strategy (8 NeuronCores, SPMD, full I/O):
  - Host folds symmetric deg^-1/2 normalization + edge weight into one
    per-edge scalar w~ = dinv[src]*w*dinv[dst]; self-loops become one
    extra "tile" per dst block (sequential z load, w~ = dinv^2).
  - Destinations partitioned contiguously: 8 cores x 98 blocks x 128.
    Sources split into 4 groups so int16 indices work with dma_gather;
    the group windows are 32768 rows (int16 range) vs 25088-row spacing,
    so groups 1-3 can shed overflow edges to the previous group. Static
    per-block tile caps (6,4,4,4) then hold w.h.p., cutting gather
    padding vs. a uniform cap. Cells are padded with dummy index-0 rows
    (weight 0) so gathers have NO -1 skips and NO runtime counts, which
    lets 7 blocks share one dma_gather call per group (amortizes the
    ~1us SWDGE descriptor-generation cost on the Pool engine that
    dominated the previous version).
  - The one-hot scatter matrices are built ON DEVICE from a compact
    (slot, w) stream (4 bytes/edge-slot instead of 256): per block, two
    wide DVE ops over [128, 19*128] fp16:
        S = (iota == slot_bcast) * w_bcast
    using step-0 broadcast access patterns; this removes the 67MB/layer
    one-hot stream from DRAM that previously half-filled the DMA bus.
  - Per dst block: PSUM aggT[f, n] += G_t.T @ S_t over 19 tiles
    (TensorE fp16, fp32 accum), then out = relu(aggT.T @ W + b).
  - Two launches (one per GCN layer) of the same compiled program.
"""

import os
from contextlib import ExitStack

import numpy as np

import concourse.bacc as bacc
import concourse.bass as bass
import concourse.mybir as mybir
import concourse.tile as tile
from concourse.tile import add_dep_helper
from concourse import bass_utils

P = 128
D = 128
NCORES = 8
NGROUP = 4
N_NODES = 100000
NB_PER_CORE = 98
SHARD = NB_PER_CORE * P          # 12544
N_PAD = SHARD * NCORES           # 100352
GROWS = N_PAD // NGROUP          # 25088 group spacing
WINDOW = 1 << 15                 # 32768 int16-addressable rows per group
ZROWS = (NGROUP - 1) * GROWS + WINDOW   # padded z rows: 108032
KBLK = 7                         # dst blocks per dma_gather call
NCALL = NB_PER_CORE // KBLK      # 14
CAPS = (5, 4, 4, 4)              # gather tiles per (block, group)
CAPS_FALLBACK = (6, 4, 4, 4)
PREP = bool(int(os.environ.get("GCN_PREP", "1")))

_nc_cache = {}


def build_nc(caps, prep):
    dt = mybir.dt
    TB = sum(caps)
    TB1 = TB + 1
    nblk = NB_PER_CORE
    ixcols = KBLK * TB * 8                 # idx cols per call (16-wrapped)
    swcols = KBLK * 2 * TB1                # slot+w cols per call
    nc = bacc.Bacc(
        "TRN2",
        target_bir_lowering=False,
        debug=False,
        enable_asserts=False,
        num_devices=1,
        num_swdge_queues=4,
    )
    zt = nc.dram_tensor("zt", [ZROWS, D], dt.float16, kind="ExternalInput")
    ixd = nc.dram_tensor("ixd", [NCALL, P, ixcols], dt.int16,
                         kind="ExternalInput")
    swd = nc.dram_tensor("swd", [NCALL, P, swcols], dt.float16,
                         kind="ExternalInput")
    zself = nc.dram_tensor("zself", [nblk * P, D], dt.float16,
                           kind="ExternalInput")
    iot = nc.dram_tensor("iot", [P, TB1 * P], dt.float16, kind="ExternalInput")
    cnt = nc.dram_tensor("cnt", [1, nblk * NGROUP], dt.int32,
                         kind="ExternalInput")
    wt = nc.dram_tensor("wt", [D, D], dt.float16, kind="ExternalInput")
    brow = nc.dram_tensor("brow", [1, D], dt.float16, kind="ExternalInput")
    out = nc.dram_tensor("out", [nblk * P, D], dt.float32,
                         kind="ExternalOutput")

    with tile.TileContext(nc) as tc, ExitStack() as ctx:
        const = ctx.enter_context(tc.tile_pool(name="const", bufs=1))
        meta = ctx.enter_context(tc.tile_pool(name="meta", bufs=3))
        swp = ctx.enter_context(tc.tile_pool(name="swp", bufs=3))
        zsp = ctx.enter_context(tc.tile_pool(name="zsp", bufs=3))
        gpools = [
            ctx.enter_context(tc.tile_pool(name=f"g{g}", bufs=4))
            for g in range(NGROUP)
        ]
        spool = ctx.enter_context(tc.tile_pool(name="s", bufs=4))
        apool = ctx.enter_context(tc.tile_pool(name="agg", bufs=3))
        opool = ctx.enter_context(tc.tile_pool(name="o", bufs=2))
        ppool = ctx.enter_context(tc.tile_pool(name="ps", bufs=3, space="PSUM"))
        p2pool = ctx.enter_context(tc.tile_pool(name="ps2", bufs=2,
                                                space="PSUM"))

        w_t = const.tile([D, D], dt.float16)
        nc.sync.dma_start(out=w_t[:], in_=wt[:])
        b_t = const.tile([1, D], dt.float16)
        nc.sync.dma_start(out=b_t[:], in_=brow[:])
        ones_t = const.tile([1, P], dt.float16)
        nc.vector.memset(ones_t[:], 1.0)
        iota_t = const.tile([P, TB1 * P], dt.float16)
        nc.sync.dma_start(out=iota_t[:], in_=iot[:])
        iota3 = iota_t[:].rearrange("p (t n) -> p t n", n=P)
        cnt_t = const.tile([1, nblk * NGROUP], dt.int32)
        nc.sync.dma_start(out=cnt_t[:], in_=cnt[:])
        GBUFS = 4
        prev_gather = None

        for c in range(NCALL):
            ix = meta.tile([P, ixcols], dt.int16, tag="ix")
            nc.sync.dma_start(out=ix[:], in_=ixd[c])
            sw_t = swp.tile([P, swcols], dt.float16, tag="sw")
            nc.scalar.dma_start(out=sw_t[:], in_=swd[c])
            zs = zsp.tile([P, KBLK * D], dt.float16, tag="zs")
            nc.sync.dma_start(
                out=zs[:].rearrange("p (k d) -> p k d", d=D),
                in_=zself[c * KBLK * P:(c + 1) * KBLK * P, :].rearrange(
                    "(k p) d -> p k d", p=P),
            )

            gcol0 = [0] * NGROUP
            acc = 0
            for g in range(NGROUP):
                gcol0[g] = acc
                acc += KBLK * caps[g] * 8

            ost = opool.tile([P, KBLK * D], dt.float32, tag="o")
            for k in range(KBLK):
                b = c * KBLK + k
                regs = [nc.gpsimd.alloc_register(f"cnt_{b}_{g}")
                        for g in range(NGROUP)]
                ld = nc.gpsimd.reg_load(
                    regs, cnt_t[0:1, b * NGROUP:(b + 1) * NGROUP])
                if prev_gather is not None:
                    add_dep_helper(ld.ins, prev_gather.ins, sync=False,
                                   reason="limit cnt register liveness")
                g_ws = []
                for g in range(NGROUP):
                    g_w = gpools[g].tile([P, caps[g] * P], dt.float16,
                                         tag=f"G{g}")
                    if b < GBUFS:
                        nc.vector.memset(g_w[:], 0.0)
                    o0 = gcol0[g] + k * caps[g] * 8
                    prev_gather = nc.gpsimd.dma_gather(
                        out_ap=g_w[:].rearrange("p (j n) -> p j n", n=P),
                        in_ap=zt[g * GROWS:g * GROWS + WINDOW, :],
                        idxs_ap=ix[:, o0:o0 + caps[g] * 8],
                        num_idxs=caps[g] * P,
                        num_idxs_reg=regs[g],
                        elem_size=P,
                        queue_num=g,
                        single_packet=False,
                    )
                    g_ws.append(g_w)
                s_t = spool.tile([P, TB1 * P], dt.float16, tag="S")
                s3 = s_t[:].rearrange("p (t n) -> p t n", n=P)
                slot_b = sw_t[:, k * 2 * TB1:k * 2 * TB1 + TB1, None] \
                    .broadcast_to((P, TB1, P))
                w_b = sw_t[:, k * 2 * TB1 + TB1:(k + 1) * 2 * TB1, None] \
                    .broadcast_to((P, TB1, P))
                nc.vector.tensor_tensor(
                    out=s3, in0=iota3, in1=slot_b,
                    op=mybir.AluOpType.is_equal)
                nc.vector.tensor_tensor(
                    out=s3, in0=s3, in1=w_b, op=mybir.AluOpType.mult)

                psum = ppool.tile([P, P], dt.float32, tag="psA")
                t = 0
                for g in range(NGROUP):
                    for j in range(caps[g]):
                        nc.tensor.matmul(
                            out=psum[:],
                            lhsT=g_ws[g][:, j * P:(j + 1) * P],
                            rhs=s_t[:, t * P:(t + 1) * P],
                            start=(t == 0),
                            stop=False,
                        )
                        t += 1
                nc.tensor.matmul(
                    out=psum[:],
                    lhsT=zs[:, k * D:(k + 1) * D],
                    rhs=s_t[:, TB * P:TB1 * P],
                    start=False, stop=True,
                )

                agg_t = apool.tile([P, P], dt.float16, tag="aggT")
                nc.scalar.activation(out=agg_t[:], in_=psum[:],
                                     func=mybir.ActivationFunctionType.Copy)

                psum2 = p2pool.tile([P, D], dt.float32, tag="psB")
                nc.tensor.matmul(out=psum2[:], lhsT=agg_t[:], rhs=w_t[:],
                                 start=True, stop=False)
                nc.tensor.matmul(out=psum2[:], lhsT=ones_t[:], rhs=b_t[:],
                                 start=False, stop=True)
                nc.scalar.activation(out=ost[:, k * D:(k + 1) * D],
                                     in_=psum2[:],
                                     func=mybir.ActivationFunctionType.Relu)

            nc.sync.dma_start(
                out=out[c * KBLK * P:(c + 1) * KBLK * P, :].rearrange(
                    "(k p) d -> p k d", p=P),
                in_=ost[:].rearrange("p (k d) -> p k d", d=D),
            )

    nc.compile()
    return nc


def preprocess(src, dst, ew, capst):
    """Host-side: edge->group assignment with overflow cascade, padded
    per-cell gather index streams, and compact (slot, w) scatter metadata.

    Returns (ixd, swd) with
      ixd: [NCORES, NCALL, P, KBLK*TB*8] int16 wrapped gather indices
      swd: [NCORES, NCALL, P, KBLK*2*TB1] fp16 slot/weight columns
    """
    TB = sum(capst)
    TB1 = TB + 1
    TBASE = (0, capst[0], capst[0] + capst[1], capst[0] + capst[1] + capst[2])
    nblocks = NCORES * NB_PER_CORE
    deg = np.bincount(dst, weights=ew.astype(np.float64),
                      minlength=N_NODES) + 1.0
    dinv = (1.0 / np.sqrt(deg)).astype(np.float32)
    wtil = (dinv[src] * ew.astype(np.float32) * dinv[dst]).astype(np.float32)
    wself = np.zeros(N_PAD, np.float32)
    wself[:N_NODES] = dinv * dinv

    blk = (dst // P).astype(np.int64)
    ng = (src // GROWS).astype(np.int64)
    zone = (src % GROWS) < (WINDOW - GROWS)
    elig = zone & (ng >= 1)

    cellng = blk * NGROUP + ng
    n = np.bincount(cellng, minlength=nblocks * NGROUP) \
        .reshape(nblocks, NGROUP)
    e = np.bincount(cellng[elig], minlength=nblocks * NGROUP) \
        .reshape(nblocks, NGROUP)

    caps = np.array(capst) * P
    shed = np.zeros((nblocks, NGROUP), np.int64)
    load = n[:, 3]
    for g in (3, 2, 1):
        shed[:, g] = np.maximum(load - caps[g], 0)
        if not (shed[:, g] <= e[:, g]).all():
            raise RuntimeError("group shed infeasible; raise CAPS")
        load = n[:, g - 1] + shed[:, g]
    if not (load <= caps[0]).all():
        raise RuntimeError("group 0 overflow; raise CAPS")

    # shed the smallest-src eligible edges of each (block, group) cell
    order = np.lexsort((src, ~elig, cellng))
    starts = np.zeros(nblocks * NGROUP, np.int64)
    counts = n.reshape(-1)
    np.cumsum(counts[:-1], out=starts[1:])
    rank = np.arange(len(src)) - starts[cellng[order]]
    shed_sorted = rank < shed.reshape(-1)[cellng[order]]
    ag = ng.copy()
    ag[order[shed_sorted]] -= 1

    # final cells, sorted by (cell, src); position within cell
    cell = blk * NGROUP + ag
    order2 = np.lexsort((src, cell))
    cell_s = cell[order2]
    src_s = src[order2]
    dst_s = dst[order2]
    w_s = wtil[order2]
    counts2 = np.bincount(cell_s, minlength=nblocks * NGROUP)
    starts2 = np.zeros(nblocks * NGROUP, np.int64)
    np.cumsum(counts2[:-1], out=starts2[1:])
    pos = np.arange(len(src_s)) - starts2[cell_s]

    ag_s = cell_s % NGROUP
    blk_s = cell_s // NGROUP
    i16 = (src_s - ag_s * GROWS).astype(np.int16)

    # padded per-group index arrays [nblocks, cap_rows]; -1 tail padding
    # (skipped by dma_gather via the runtime count register)
    cnt = counts2.reshape(nblocks, NGROUP).astype(np.int32)
    idxpads = []
    for g in range(NGROUP):
        m = ag_s == g
        a = np.full((nblocks, caps[g]), -1, np.int16)
        a[blk_s[m], pos[m]] = i16[m]
        # >= 1 valid index per cell (dummy idx 0, weight 0)
        empty = cnt[:, g] == 0
        a[empty, 0] = 0
        idxpads.append(a)
    cnt = np.maximum(cnt, 1)
    cnt = np.ascontiguousarray(
        cnt.reshape(NCORES, 1, NB_PER_CORE * NGROUP))

    # slot/w columns [nblocks, P, TB1]
    slot = np.zeros((nblocks, P, TB1), np.float16)
    warr = np.zeros((nblocks, P, TB1), np.float16)
    tcol = np.take(TBASE, ag_s) + pos // P
    prow = pos % P
    slot[blk_s, prow, tcol] = (dst_s % P).astype(np.float16)
    warr[blk_s, prow, tcol] = w_s.astype(np.float16)
    slot[:, :, TB] = np.arange(P, dtype=np.float16)[None, :]
    warr[:, :, TB] = wself.astype(np.float16).reshape(nblocks, P)

    # wrap indices: per (core, call, group): [KBLK*cap] -> [128, KBLK*cap/16]
    ixparts = []
    for g in range(NGROUP):
        a = idxpads[g].reshape(NCORES, NCALL, KBLK * caps[g])
        a = a.reshape(NCORES, NCALL, KBLK * caps[g] // 16, 16)
        a = a.transpose(0, 1, 3, 2)                   # [8, 14, 16, cols]
        a = np.tile(a, (1, 1, 8, 1))                  # [8, 14, 128, cols]
        ixparts.append(a)
    ixd = np.ascontiguousarray(np.concatenate(ixparts, axis=3))

    sw = np.concatenate([slot, warr], axis=2)         # [nblocks, P, 2*TB1]
    sw = sw.reshape(NCORES, NCALL, KBLK, P, 2 * TB1)
    swd = np.ascontiguousarray(sw.transpose(0, 1, 3, 2, 4).reshape(
        NCORES, NCALL, P, KBLK * 2 * TB1))
    return ixd, swd, cnt


def run_layer(nc, z_f16, ixd, swd, cnt, W, b, tb1, *, trace=False,
              tmpdir=None):
    iot = np.tile(np.arange(P, dtype=np.float16), (P, tb1)) \
        .reshape(P, tb1 * P)
    in_maps = []
    for c in range(NCORES):
        in_maps.append({
            "zt": z_f16,
            "zself": z_f16[c * SHARD:(c + 1) * SHARD],
            "ixd": ixd[c],
            "swd": swd[c],
            "cnt": cnt[c],
            "iot": iot,
            "wt": np.ascontiguousarray(W.astype(np.float16)),
            "brow": np.ascontiguousarray(
                b.astype(np.float16).reshape(1, D)),
        })
    res = bass_utils.run_bass_kernel_spmd(
        nc, in_maps, core_ids=list(range(NCORES)), trace=trace, tmpdir=tmpdir,
    )
    out = np.concatenate([res.results[c]["out"] for c in range(NCORES)],
                         axis=0)
    return out, res


def _enable_tracing():
    """Install the NTFF profile hook that this image's antenv lacks, and
    neuter the artifact upload (no bucket access here)."""
    import sys
    import types
    try:
        import antenv.axon_hooks  # noqa: F401
        have = True
    except ImportError:
        have = False
    if not have:
        mod = types.ModuleType("antenv.axon_hooks")
        mod._hook = None

        def set_axon_ntff_profile_hook(h):
            mod._hook = h

        def get_axon_ntff_profile_hook():
            return mod._hook

        mod.set_axon_ntff_profile_hook = set_axon_ntff_profile_hook
        mod.get_axon_ntff_profile_hook = get_axon_ntff_profile_hook
        sys.modules["antenv.axon_hooks"] = mod
        from trn_agent_boot.trn_boot import _ntff_profile_via_ctypes
        hook = _ntff_profile_via_ctypes("/opt/axon/libaxon_pjrt.so")
        mod.set_axon_ntff_profile_hook(hook)
    bass_utils.upload_artifacts = lambda tmpdir: f"local:{tmpdir}"


def kernel(x, edge_index, edge_weight, W1, b1, W2, b2):
    x = np.asarray(x, dtype=np.float32)
    edge_index = np.asarray(edge_index)
    edge_weight = np.asarray(edge_weight, dtype=np.float32)
    src = edge_index[0].astype(np.int64)
    dst = edge_index[1].astype(np.int64)

    try:
        capst = CAPS
        ixd, swd, cnt = preprocess(src, dst, edge_weight, capst)
    except RuntimeError:
        capst = CAPS_FALLBACK
        ixd, swd, cnt = preprocess(src, dst, edge_weight, capst)
    tb1 = sum(capst) + 1

    key = (capst, PREP)
    if key not in _nc_cache:
        _nc_cache[key] = build_nc(capst, PREP)
    nc = _nc_cache[key]

    trace = bool(int(os.environ.get("GCN_TRACE", "0")))
    if trace:
        _enable_tracing()

    z1 = np.zeros((ZROWS, D), np.float16)
    z1[:N_NODES] = x.astype(np.float16)
    h1, res1 = run_layer(nc, z1, ixd, swd, cnt, W1, b1, tb1, trace=trace)

    z2 = np.zeros((ZROWS, D), np.float16)
    z2[:N_PAD] = h1.astype(np.float16)
    h2, res2 = run_layer(nc, z2, ixd, swd, cnt, W2, b2, tb1, trace=trace)

    if trace:
        t1 = res1.exec_time_ns or 0
        t2 = res2.exec_time_ns or 0
        print(f"[kernel] layer1 exec: {t1} ns, layer2 exec: {t2} ns, "
              f"total: {t1 + t2} ns")
        kernel.last_exec_ns = t1 + t2
        kernel.last_results = (res1, res2)

    return h2[:N_NODES].astype(np.float32)
